# revision 1
# baseline (speedup 1.0000x reference)
"""Based-style linear attention (Taylor feature map) on 8 Trainium2 cores.

Math: reference computes, per head h (FDIM=16, HEAD_DIM=64):
    q,k = HS@Wq, HS@Wk    (per-head 16 dims), v = HS@Wv (per-head 64 dims)
    phi(x) = [1, x/2, outer(x,x)/(sqrt(2)*4)]      (273 dims)
    y_t = sum_{s<=t} (phi(q_t).phi(k_s)) v_s / sum_{s<=t} phi(q_t).phi(k_s)
    out = concat_h(y) @ Wo

Key identity: phi(q).phi(k) = 1 + S/4 + S^2/32 where S = q.k (16-dim dot)
            = Square(S/sqrt(32) + 1/sqrt(2)) + 1/2.
So scores come from 16-dim dot products + one ACT Square pass; the 273-dim
feature map is never materialized.

Sharding: head-parallel, no collectives. 16 virtual heads (12 real + 4
zero dummies), 2 per core. Each core: q/k/v projections for its heads
(full L=1024), causal chunked attention (8 chunks of 128), partial
output y_heads @ Wo_rows. Host sums the 8 partial outputs.

Layout is chosen so every matmul has a large free dim (N up to 512):
 - scores: per kv-chunk j, one matmul produces S^T[kv=128, t=j*128..1024]
   for all later query positions at once (k as stationary operand).
 - A@V: V-as-stationary, rhs = Square(S^T) big tile -> accumulates
   num^T[vc=65, t=0..1024] across j. No transposes anywhere.
 - the "+1/2" in every causal score is folded in via constant-matrix
   matmuls (htri for the diagonal chunk, sel x colsum for prior chunks);
   only diagonal 128x128 blocks need an elementwise tri-mask.
 - den rides along as v-column 64 (ones), y = num * (1/den) with the
   reciprocal row broadcast across partitions by a K=1 matmul.

Matmul operands are bf16 (PE streams 2 B/lane/cycle, so bf16 is 2x fp32;
products of bf16 pairs accumulate exactly in fp32 PSUM). PSUM, den and
the reciprocal stay fp32.
"""

import math

import numpy as np
import ml_dtypes

import concourse.bass as bass
import concourse.mybir as mybir
import concourse.tile as tile
from concourse import bacc
from concourse.bass_utils import run_bass_kernel_spmd

L = 1024
D = 768
H = 12
FD = 16
HD = 64
NCORE = 8
NCH = 8  # L chunks of 128
KB = 6  # contraction blocks of 128 over D
F32 = mybir.dt.float32
BF16 = mybir.dt.bfloat16

# dtype knobs (bf16 = 2x PE throughput; flip to F32 to trade speed for bits)
DT_PROJ = BF16
DT_ATT = BF16
DT_OUT = BF16

A_SCALE = 1.0 / math.sqrt(32.0)
A_BIAS = 1.0 / math.sqrt(2.0)

_compiled_nc = None
_last_in_maps = None


def _np_dt(dt):
    return ml_dtypes.bfloat16 if dt == BF16 else np.float32


def _bank_splits(lo, hi, bank=512):
    """Split [lo, hi) at multiples of `bank` (PSUM bank boundaries)."""
    out = []
    a = lo
    while a < hi:
        b = min(hi, (a // bank + 1) * bank)
        out.append((a, b))
        a = b
    return out


def _build_nc():
    nc = bacc.Bacc("TRN2", target_bir_lowering=False, debug=False, num_devices=NCORE)

    hsT = nc.dram_tensor("hsT", [D, L], DT_PROJ, kind="ExternalInput")
    wqv = nc.dram_tensor("wqv", [D, 258], DT_PROJ, kind="ExternalInput")
    wo = nc.dram_tensor("wo", [128, D], DT_OUT, kind="ExternalInput")
    # consts packed: tri 0:128 | htri 128:256 | ones8 256:320 | sel 320:1344
    c_all = nc.dram_tensor("c_all", [128, 1344], DT_ATT, kind="ExternalInput")
    out = nc.dram_tensor("out", [L, D], DT_OUT, kind="ExternalOutput")

    with tile.TileContext(nc) as tc:
        with (
            tc.tile_pool(name="cst", bufs=1) as cst,
            tc.tile_pool(name="sqp", bufs=4) as sqp,
            tc.tile_pool(name="wrk", bufs=2) as wrk,
        ):
            # ---- load inputs to SBUF, ordered so the first projection
            # matmul (needs wqv kb=0 + hs kb=0) can start asap ----
            wqv_re = wqv.ap().rearrange("(po pi) f -> pi po f", pi=128)
            hs_re = hsT.ap().rearrange("(po pi) f -> pi po f", pi=128)
            wqv_sb = cst.tile([128, KB, 258], DT_PROJ, tag="wqv")
            hs_sb = [
                cst.tile([128, L], DT_PROJ, tag=f"hs{kb}", name=f"hs{kb}")
                for kb in range(KB)
            ]
            for kb in range(KB):
                nc.sync.dma_start(out=wqv_sb[:, kb, :], in_=wqv_re[:, kb, :])
                nc.sync.dma_start(out=hs_sb[kb], in_=hs_re[:, kb, :])
            wk_sb = wqv_sb[:, :, 0:64]
            wq_sb = wqv_sb[:, :, 64:128]
            wv_sb = wqv_sb[:, :, 128:258]
            call_sb = cst.tile([128, 1344], DT_ATT, tag="call")
            nc.sync.dma_start(out=call_sb, in_=c_all.ap())
            tri_sb = call_sb[:, 0:128]
            htri_sb = call_sb[:, 128:256]
            ones8_sb = call_sb[:, 256:320]
            sel_sb = call_sb[0:8, 320:1344]
            # wo split into per-head tiles so o-proj operands share base 0
            wo_sb = []
            for h in range(2):
                t = cst.tile([64, D], DT_OUT, tag=f"wo{h}", name=f"wo{h}")
                nc.sync.dma_start(out=t, in_=wo.ap()[64 * h : 64 * h + 64, :])
                wo_sb.append(t)
            bias_sb = cst.tile([128, 1], F32, tag="bias")
            nc.vector.memset(bias_sb, A_BIAS)
            # row of ones at partition 64, for the den-reciprocal broadcast
            ones64_sb = cst.tile([65, 64], F32, tag="ones64")
            nc.vector.memset(ones64_sb, 0.0)
            nc.vector.memset(ones64_sb[64:65, :], 1.0)

            kq_sb = cst.tile([64, 2048], DT_ATT, tag="kq")
            vx_sb = cst.tile([128, NCH, 130], DT_ATT, tag="vx")
            colsum_sb = cst.tile([8, 130], DT_ATT, tag="colsum")

            # ================= projections =================
            with tc.tile_pool(name="ps1", bufs=3, space="PSUM") as ps1:
                # q/k -> kq_sb [64, 2048]; partitions 0-15 head0, 32-47 head1
                # (rest zero); cols 0-1023 = k^T, 1024-2047 = q^T
                for w_sb, coff in ((wk_sb, 0), (wq_sb, 1024)):
                    for half in range(2):
                        p = ps1.tile([64, 512], F32, tag="pB", name=f"pqk{coff}_{half}")
                        for kb in range(KB):
                            nc.tensor.matmul(
                                p,
                                w_sb[:, kb, :],
                                hs_sb[kb][:, half * 512 : (half + 1) * 512],
                                start=(kb == 0),
                                stop=(kb == KB - 1),
                            )
                        nc.vector.tensor_copy(
                            kq_sb[:, coff + half * 512 : coff + (half + 1) * 512], p
                        )
                # v -> vx_sb [128, 8, 130]: cols 0-63 v_h0, 64 ones,
                # 65-128 v_h1, 129 ones
                for ch in range(NCH):
                    pv = ps1.tile([128, 130], F32, tag="pB", name=f"pv{ch}")
                    for kb in range(KB):
                        nc.tensor.matmul(
                            pv,
                            hs_sb[kb][:, ch * 128 : (ch + 1) * 128],
                            wv_sb[:, kb, :],
                            start=(kb == 0),
                            stop=(kb == KB - 1),
                        )
                    nc.vector.tensor_copy(vx_sb[:, ch, :], pv)
                nc.vector.memset(vx_sb[:, :, 64], 1.0)
                nc.vector.memset(vx_sb[:, :, 129], 1.0)

                # per-chunk column sums of vx (inter-chunk +1/2 term)
                pcs = ps1.tile([8, 130], F32, tag="pB", name="pcs")
                for ch in range(NCH):
                    nc.tensor.matmul(
                        pcs,
                        ones8_sb[:, ch * 8 : (ch + 1) * 8],
                        vx_sb[:, ch, :],
                        start=(ch == 0),
                        stop=(ch == NCH - 1),
                    )
                nc.vector.tensor_copy(colsum_sb, pcs)

            # ================= attention =================
            yT_sb = [
                cst.tile([64, L], DT_OUT, tag=f"yT{h}", name=f"yT{h}") for h in range(2)
            ]
            with tc.tile_pool(name="psnum", bufs=1, space="PSUM") as psnum:
                nums = [
                    psnum.tile([65, L], F32, tag=f"pN{h}", name=f"num{h}")
                    for h in range(2)
                ]
                with tc.tile_pool(name="psa", bufs=2, space="PSUM") as psa:
                    for j in range(NCH):
                        tlo = j * 128
                        width = L - tlo
                        for h in range(2):
                            pa = psa.tile([128, 1024], F32, tag="pA", name=f"pa{j}_{h}")[
                                :, :width
                            ]
                            for a, b in _bank_splits(0, width):
                                nc.tensor.matmul(
                                    pa[:, a:b],
                                    kq_sb[32 * h : 32 * h + 32, tlo : tlo + 128],
                                    kq_sb[
                                        32 * h : 32 * h + 32,
                                        1024 + tlo + a : 1024 + tlo + b,
                                    ],
                                    start=True,
                                    stop=True,
                                )
                            sq = sqp.tile([128, 1024], DT_ATT, tag="sq", name=f"sq{j}_{h}")[
                                :, :width
                            ]
                            nc.scalar.activation(
                                out=sq,
                                in_=pa,
                                func=mybir.ActivationFunctionType.Square,
                                scale=A_SCALE,
                                bias=bias_sb,
                            )
                            # mask the diagonal block (i == j)
                            nc.vector.tensor_mul(sq[:, 0:128], sq[:, 0:128], tri_sb)
                            # num^T += V_j^T-stationary @ sq
                            for a, b in _bank_splits(tlo, L):
                                nc.tensor.matmul(
                                    nums[h][:, a:b],
                                    vx_sb[:, j, 65 * h : 65 * h + 65],
                                    sq[:, a - tlo : b - tlo],
                                    start=(j == 0),
                                    stop=False,
                                )
                    # intra-chunk +1/2 term: 0.5 * prefix-sums of V_i
                    for h in range(2):
                        for i in range(NCH):
                            nc.tensor.matmul(
                                nums[h][:, i * 128 : (i + 1) * 128],
                                vx_sb[:, i, 65 * h : 65 * h + 65],
                                htri_sb,
                                start=False,
                                stop=False,
                            )
                        # inter-chunk +1/2 term: 0.5 * sum of prior colsums
                        for a, b in _bank_splits(0, L):
                            nc.tensor.matmul(
                                nums[h][:, a:b],
                                colsum_sb[:, 65 * h : 65 * h + 65],
                                sel_sb[:, a:b],
                                start=False,
                                stop=True,
                            )

                # y^T = num^T[0:64] / den  (den = row 64)
                with tc.tile_pool(name="ps2", bufs=2, space="PSUM") as ps2:
                    for h in range(2):
                        # custom-DVE ops require base partition 0: run the
                        # approx reciprocal over the whole tile (rows 0-63
                        # are garbage, only den row 64 is used)
                        rc = wrk.tile([65, L], F32, tag="rc")
                        nc.vector.reciprocal_approx_fast(out=rc, in_=nums[h])
                        prb = ps2.tile([64, L], F32, tag="prb", name=f"prb{h}")
                        for a, b in _bank_splits(0, L):
                            nc.tensor.matmul(
                                prb[:, a:b],
                                ones64_sb[64:65, :],
                                rc[64:65, a:b],
                                start=True,
                                stop=True,
                            )
                        rb = wrk.tile([64, L], F32, tag="rb")
                        nc.any.tensor_copy(rb, prb)
                        nc.vector.tensor_mul(yT_sb[h], nums[h][0:64, :], rb)

            # ================= output projection =================
            with tc.tile_pool(name="ps3", bufs=4, space="PSUM") as ps3:
                for i in range(NCH):
                    osb = wrk.tile([128, D], DT_OUT, tag="osb")
                    for a, b in ((0, 512), (512, 768)):
                        po = ps3.tile([128, 512], F32, tag="po", name=f"po{i}_{a}")[
                            :, : b - a
                        ]
                        for h in range(2):
                            nc.tensor.matmul(
                                po,
                                yT_sb[h][:, i * 128 : (i + 1) * 128],
                                wo_sb[h][:, a:b],
                                start=(h == 0),
                                stop=(h == 1),
                            )
                        nc.any.tensor_copy(osb[:, a:b], po)
                    nc.sync.dma_start(out=out.ap()[i * 128 : (i + 1) * 128, :], in_=osb)

    nc.finalize()
    return nc


def _host_consts():
    s = np.arange(128)[:, None]
    t = np.arange(128)[None, :]
    tri = (s <= t).astype(np.float32)
    htri = 0.5 * tri
    sel = np.zeros((8, 1024), dtype=np.float32)
    for i in range(8):
        sel[:i, i * 128 : (i + 1) * 128] = 0.5
    ones8 = np.zeros((128, 64), dtype=np.float32)
    for ch in range(8):
        ones8[:, ch * 8 + ch] = 1.0
    return tri, htri, sel, ones8


def kernel(hidden_states, Wq, Wk, Wv, Wo):
    global _compiled_nc, _last_in_maps
    hs = np.asarray(hidden_states, dtype=np.float32)[0]  # [L, D]
    Wq = np.asarray(Wq, dtype=np.float32)
    Wk = np.asarray(Wk, dtype=np.float32)
    Wv = np.asarray(Wv, dtype=np.float32)
    Wo = np.asarray(Wo, dtype=np.float32)

    if _compiled_nc is None:
        _compiled_nc = _build_nc()
    nc = _compiled_nc

    proj_dt = _np_dt(DT_PROJ)
    att_dt = _np_dt(DT_ATT)
    out_dt = _np_dt(DT_OUT)

    hsT = np.ascontiguousarray(hs.T).astype(proj_dt)  # [D, L]
    tri, htri, sel, ones8 = _host_consts()
    c_all = np.zeros((128, 1344), dtype=np.float32)
    c_all[:, 0:128] = tri
    c_all[:, 128:256] = htri
    c_all[:, 256:320] = ones8
    c_all[0:8, 320:1344] = sel
    c_all = c_all.astype(att_dt)

    in_maps = []
    for c in range(NCORE):
        heads = [2 * c, 2 * c + 1]
        wk_c = np.zeros((D, 64), dtype=np.float32)
        wq_c = np.zeros((D, 64), dtype=np.float32)
        wv_c = np.zeros((D, 130), dtype=np.float32)
        wo_c = np.zeros((128, D), dtype=np.float32)
        for hi, h in enumerate(heads):
            if h >= H:
                continue
            wk_c[:, 32 * hi : 32 * hi + FD] = Wk[:, h * FD : (h + 1) * FD]
            wq_c[:, 32 * hi : 32 * hi + FD] = Wq[:, h * FD : (h + 1) * FD]
            wv_c[:, 65 * hi : 65 * hi + HD] = Wv[:, h * HD : (h + 1) * HD]
            wo_c[64 * hi : 64 * hi + HD, :] = Wo[h * HD : (h + 1) * HD, :]
        wqv_c = np.concatenate([wk_c, wq_c, wv_c], axis=1)
        in_maps.append(
            {
                "hsT": hsT,
                "wqv": wqv_c.astype(proj_dt),
                "wo": wo_c.astype(out_dt),
                "c_all": c_all,
            }
        )

    _last_in_maps = in_maps
    res = run_bass_kernel_spmd(nc, in_maps, list(range(NCORE)))
    acc = np.zeros((L, D), dtype=np.float32)
    for c in range(NCORE):
        acc += np.asarray(res.results[c]["out"], dtype=np.float32)
    return acc.reshape(1, L, D)



# revision 2
# speedup vs baseline: 1.0244x; 1.0244x over previous
"""Based-style linear attention (Taylor feature map) on 8 Trainium2 cores.

Math: reference computes, per head h (FDIM=16, HEAD_DIM=64):
    q,k = HS@Wq, HS@Wk    (per-head 16 dims), v = HS@Wv (per-head 64 dims)
    phi(x) = [1, x/2, outer(x,x)/(sqrt(2)*4)]      (273 dims)
    y_t = sum_{s<=t} (phi(q_t).phi(k_s)) v_s / sum_{s<=t} phi(q_t).phi(k_s)
    out = concat_h(y) @ Wo

Key identity: phi(q).phi(k) = 1 + S/4 + S^2/32 where S = q.k (16-dim dot)
            = Square(S/sqrt(32) + 1/sqrt(2)) + 1/2.
So scores come from 16-dim dot products + one ACT Square pass; the 273-dim
feature map is never materialized.

Sharding: head-parallel, no collectives. 16 virtual heads (12 real + 4
zero dummies), 2 per core. Host sums the 8 partial outputs.

v2 structure (all per core, 2 heads):
 - inputs packed into one [128, NWIN] weights/consts tensor (2 DMAs) +
   hsT in 6 per-kb DMAs, issued in parallel from the two HWDGE queues
   (sync + scalar) so descriptor-issue latency doesn't serialize.
 - 8 dummy matmuls on zeros at kernel start warm the PE HAM clock gate
   (cold 1.2GHz -> warm 2.4GHz) while input DMA streams in.
 - q/k projections use ONE merged 128-col stationary (k_h0|k_h1|q_h0|q_h1
   16-col groups at 32-col offsets) so hs is streamed once, not twice.
 - score matmuls have K=32: the two heads' stationaries sit at partition
   offsets 0/32 so they row-tile into independent 32x128 PE sub-arrays
   and run concurrently.
 - squares on ACT: (N+352)/1.2 ns per call, so the two heads' tiles for
   j>=4 share one psum bank pair and one ACT call.
 - nums[h] [65, L] psum: v-cols 0-63 + den ride-along col 64 (ones), +1/2
   causal terms folded in via htri / colsum-sel constant matmuls.
 - divide: reciprocal_approx_fast on nums, K=1 matmul broadcasts the den
   reciprocal rows into a [128, L] psum, one ACT copy -> rb, two DVE muls
   produce yT [128, L] with h0 in partitions 0-63, h1 in 64-127.
 - o-proj: yT stacked layout makes it ONE K=128 matmul group per chunk
   (wo is [128, 768] with both heads' rows) - half the streaming of a
   per-head accumulation. psum->sbuf copies alternate ACT/DVE; output
   chunks pair up into 4 big DMAs split across both queues.
"""

import math

import numpy as np
import ml_dtypes

import concourse.bass as bass
import concourse.mybir as mybir
import concourse.tile as tile
from concourse import bacc
from concourse.bass_utils import run_bass_kernel_spmd

L = 1024
D = 768
H = 12
FD = 16
HD = 64
NCORE = 8
NCH = 8  # L chunks of 128
KB = 6  # contraction blocks of 128 over D
F32 = mybir.dt.float32
BF16 = mybir.dt.bfloat16
DT = BF16

A_SCALE = 1.0 / math.sqrt(32.0)
A_BIAS = 1.0 / math.sqrt(2.0)

# win column map ([128, NWIN] bf16)
WQK0 = 0                 # 6 kb-blocks x 128 (merged qk stationary)
WV0 = WQK0 + KB * 128    # 6 kb-blocks x 130
WO0 = WV0 + KB * 130     # wo [128, 768] (h0 rows 0-63, h1 rows 64-127)
TRI2_0 = WO0 + D         # [tri | tri] 256
HTRI0 = TRI2_0 + 256     # htri 128
ONES8_0 = HTRI0 + 128    # ones8 64
NWIN = ONES8_0 + 64
WINA = WO0               # split point of the two win DMAs

_compiled_nc = None
_last_in_maps = None


def _bank_splits(lo, hi, bank=512):
    """Split [lo, hi) at multiples of `bank` (PSUM bank boundaries)."""
    out = []
    a = lo
    while a < hi:
        b = min(hi, (a // bank + 1) * bank)
        out.append((a, b))
        a = b
    return out


def _build_nc():
    nc = bacc.Bacc("TRN2", target_bir_lowering=False, debug=False, num_devices=NCORE)

    hsT = nc.dram_tensor("hsT", [D, L], DT, kind="ExternalInput")
    win = nc.dram_tensor("win", [128, NWIN], DT, kind="ExternalInput")
    selw = nc.dram_tensor("selw", [8, L], DT, kind="ExternalInput")
    outp = nc.dram_tensor("outp", [L, D], DT, kind="ExternalOutput")

    with tile.TileContext(nc) as tc:
        with (
            tc.tile_pool(name="cst", bufs=1) as cst,
            tc.tile_pool(name="sqp", bufs=3) as sqp,
            tc.tile_pool(name="wrk", bufs=2) as wrk,
        ):
            # ---- input DMAs: split across the two HWDGE queues ----
            hs_re = hsT.ap().rearrange("(po pi) f -> pi po f", pi=128)
            win_sb = cst.tile([128, NWIN], DT, tag="win")
            sel_sb = cst.tile([8, L], DT, tag="sel")
            hs_sb = [
                cst.tile([128, L], DT, tag=f"hs{kb}", name=f"hs{kb}")
                for kb in range(KB)
            ]
            # sync queue: win_a, hs0, hs2, hs4
            nc.sync.dma_start(out=win_sb[:, 0:WINA], in_=win.ap()[:, 0:WINA])
            nc.sync.dma_start(out=hs_sb[0], in_=hs_re[:, 0, :])
            nc.sync.dma_start(out=hs_sb[2], in_=hs_re[:, 2, :])
            nc.sync.dma_start(out=hs_sb[4], in_=hs_re[:, 4, :])
            # scalar queue: selw, win_b, hs1, hs3, hs5
            nc.scalar.dma_start(out=sel_sb, in_=selw.ap())
            nc.scalar.dma_start(out=win_sb[:, WINA:NWIN], in_=win.ap()[:, WINA:NWIN])
            nc.scalar.dma_start(out=hs_sb[1], in_=hs_re[:, 1, :])
            nc.scalar.dma_start(out=hs_sb[3], in_=hs_re[:, 3, :])
            nc.scalar.dma_start(out=hs_sb[5], in_=hs_re[:, 5, :])

            def wqk(kb):
                return win_sb[:, WQK0 + kb * 128 : WQK0 + (kb + 1) * 128]

            def wv(kb):
                return win_sb[:, WV0 + kb * 130 : WV0 + (kb + 1) * 130]

            wo_sb = win_sb[:, WO0 : WO0 + D]
            tri2_sb = win_sb[:, TRI2_0 : TRI2_0 + 256]
            htri_sb = win_sb[:, HTRI0 : HTRI0 + 128]
            ones8_sb = win_sb[:, ONES8_0 : ONES8_0 + 64]

            bias_sb = cst.tile([128, 1], F32, tag="bias")
            nc.vector.memset(bias_sb, A_BIAS)
            # row of ones at partition 64, for the den-reciprocal broadcast
            ones64_sb = cst.tile([65, 64], F32, tag="ones64")
            nc.vector.memset(ones64_sb, 0.0)
            nc.vector.memset(ones64_sb[64:65, :], 1.0)

            kq_sb = cst.tile([64, 2048], DT, tag="kq")
            vx_sb = cst.tile([128, NCH, 130], DT, tag="vx")
            colsum_sb = cst.tile([8, 130], DT, tag="colsum")
            warm_sb = cst.tile([128, 512], DT, tag="warm")
            nc.vector.memset(warm_sb, 0.0)
            warm_out = cst.tile([128, 1], F32, tag="warmout")

            # ================= projections =================
            with tc.tile_pool(name="ps1", bufs=4, space="PSUM") as ps1:
                # q/k -> kq_sb [64, 2048]; partitions 0-15 head0, 32-47 head1
                # (rest zero); cols 0-1023 = k^T, 1024-2047 = q^T
                for half in range(2):
                    ph = ps1.tile([128, 512], F32, tag="pB", name=f"pqk{half}")
                    for kb in range(KB):
                        nc.tensor.matmul(
                            ph,
                            wqk(kb),
                            hs_sb[kb][:, half * 512 : (half + 1) * 512],
                            start=(kb == 0),
                            stop=(kb == KB - 1),
                        )
                    co = half * 512
                    nc.scalar.activation(
                        out=kq_sb[:, co : co + 512],
                        in_=ph[0:64, :],
                        func=mybir.ActivationFunctionType.Copy,
                    )
                    nc.vector.tensor_copy(
                        kq_sb[:, 1024 + co : 1024 + co + 512], ph[64:128, :]
                    )
                # v -> vx_sb [128, 8, 130]: cols 0-63 v_h0, 64 ones,
                # 65-128 v_h1, 129 ones
                for ch in range(NCH):
                    pv = ps1.tile([128, 130], F32, tag="pB", name=f"pv{ch}")
                    for kb in range(KB):
                        nc.tensor.matmul(
                            pv,
                            hs_sb[kb][:, ch * 128 : (ch + 1) * 128],
                            wv(kb),
                            start=(kb == 0),
                            stop=(kb == KB - 1),
                        )
                    nc.vector.tensor_copy(vx_sb[:, ch, :], pv)
                nc.vector.memset(vx_sb[:, :, 64], 1.0)
                nc.vector.memset(vx_sb[:, :, 129], 1.0)

                # per-chunk column sums of vx (inter-chunk +1/2 term)
                pcs = ps1.tile([8, 130], F32, tag="pB", name="pcs")
                for ch in range(NCH):
                    nc.tensor.matmul(
                        pcs,
                        ones8_sb[:, ch * 8 : (ch + 1) * 8],
                        vx_sb[:, ch, :],
                        start=(ch == 0),
                        stop=(ch == NCH - 1),
                    )
                nc.vector.tensor_copy(colsum_sb, pcs)

            # warm-up: keep the PE HAM busy while input DMA streams in.
            with tc.tile_pool(name="psw", bufs=1, space="PSUM") as psw:
                pw = psw.tile([128, 512], F32, tag="pw")
                for i in range(8):
                    nc.tensor.matmul(
                        pw,
                        warm_sb[:, 0:128],
                        warm_sb,
                        start=(i == 0),
                        stop=(i == 7),
                    )
                nc.vector.tensor_copy(warm_out, pw[:, 0:1])

            # ================= attention =================
            yT_sb = cst.tile([128, L], DT, tag="yT")
            with tc.tile_pool(name="psnum", bufs=1, space="PSUM") as psnum:
                nums = [
                    psnum.tile([65, L], F32, tag=f"pN{h}", name=f"num{h}")
                    for h in range(2)
                ]
                with tc.tile_pool(name="psa", bufs=2, space="PSUM") as psa:
                    for j in range(NCH):
                        tlo = j * 128
                        width = L - tlo
                        if width > 512:
                            # separate per-head psum tiles + ACT calls
                            sq = sqp.tile(
                                [128, 2, L], DT, tag="sq", name=f"sq{j}"
                            )
                            for h in range(2):
                                pa = psa.tile(
                                    [128, 1024], F32, tag="pA", name=f"pa{j}_{h}"
                                )[:, :width]
                                for a, b in _bank_splits(0, width):
                                    nc.tensor.matmul(
                                        pa[:, a:b],
                                        kq_sb[32 * h : 32 * h + 32, tlo : tlo + 128],
                                        kq_sb[
                                            32 * h : 32 * h + 32,
                                            1024 + tlo + a : 1024 + tlo + b,
                                        ],
                                        start=True,
                                        stop=True,
                                    )
                                nc.scalar.activation(
                                    out=sq[:, h, :width],
                                    in_=pa,
                                    func=mybir.ActivationFunctionType.Square,
                                    scale=A_SCALE,
                                    bias=bias_sb,
                                )
                        else:
                            # both heads share one bank pair + one ACT call
                            sq = sqp.tile(
                                [128, 2, L], DT, tag="sq", name=f"sq{j}"
                            )
                            pa = psa.tile(
                                [128, 2, 512], F32, tag="pA", name=f"pa{j}"
                            )
                            for h in range(2):
                                nc.tensor.matmul(
                                    pa[:, h, :width],
                                    kq_sb[32 * h : 32 * h + 32, tlo : tlo + 128],
                                    kq_sb[
                                        32 * h : 32 * h + 32,
                                        1024 + tlo : 1024 + tlo + width,
                                    ],
                                    start=True,
                                    stop=True,
                                )
                            nc.scalar.activation(
                                out=sq[:, :, :width],
                                in_=pa[:, :, :width],
                                func=mybir.ActivationFunctionType.Square,
                                scale=A_SCALE,
                                bias=bias_sb,
                            )
                        # mask the diagonal blocks (both heads, one op)
                        nc.vector.tensor_mul(
                            sq[:, :, 0:128],
                            sq[:, :, 0:128],
                            tri2_sb,
                        )
                        # num^T += V_j^T-stationary @ sq
                        for h in range(2):
                            for a, b in _bank_splits(tlo, L):
                                nc.tensor.matmul(
                                    nums[h][:, a:b],
                                    vx_sb[:, j, 65 * h : 65 * h + 65],
                                    sq[:, h, a - tlo : b - tlo],
                                    start=(j == 0),
                                    stop=False,
                                )
                    # intra-chunk +1/2 term: 0.5 * prefix-sums of V_i
                    for h in range(2):
                        for i in range(NCH):
                            nc.tensor.matmul(
                                nums[h][:, i * 128 : (i + 1) * 128],
                                vx_sb[:, i, 65 * h : 65 * h + 65],
                                htri_sb,
                                start=False,
                                stop=False,
                            )
                        # inter-chunk +1/2 term: 0.5 * sum of prior colsums
                        for a, b in _bank_splits(0, L):
                            nc.tensor.matmul(
                                nums[h][:, a:b],
                                colsum_sb[:, 65 * h : 65 * h + 65],
                                sel_sb[:, a:b],
                                start=False,
                                stop=True,
                            )

                # y^T = num^T[0:64] / den  (den = row 64)
                with tc.tile_pool(name="ps2", bufs=1, space="PSUM") as ps2:
                    prb = ps2.tile([128, L], F32, tag="prb")
                    rb = wrk.tile([128, L], F32, tag="rb")
                    for h in range(2):
                        # custom-DVE ops require base partition 0: run the
                        # approx reciprocal over the whole tile (rows 0-63
                        # are garbage, only den row 64 is used)
                        rc = wrk.tile([65, L], F32, tag="rc", name=f"rc{h}")
                        nc.vector.reciprocal_approx_fast(out=rc, in_=nums[h])
                        for a, b in _bank_splits(0, L):
                            nc.tensor.matmul(
                                prb[64 * h : 64 * h + 64, a:b],
                                ones64_sb[64:65, :],
                                rc[64:65, a:b],
                                start=True,
                                stop=True,
                            )
                    nc.scalar.activation(
                        out=rb, in_=prb, func=mybir.ActivationFunctionType.Copy
                    )
                    for h in range(2):
                        nc.vector.tensor_mul(
                            yT_sb[64 * h : 64 * h + 64, :],
                            nums[h][0:64, :],
                            rb[64 * h : 64 * h + 64, :],
                        )

            # ================= output projection =================
            out_re = outp.ap().rearrange("(c p) d -> p c d", p=128)
            with tc.tile_pool(name="ps3", bufs=4, space="PSUM") as ps3:
                for p in range(NCH // 2):
                    osb = wrk.tile([128, 2, D], DT, tag="osb", name=f"osb{p}")
                    for s in range(2):
                        i = 2 * p + s
                        for a, b in ((0, 512), (512, 768)):
                            po = ps3.tile([128, 512], F32, tag="po", name=f"po{i}_{a}")[
                                :, : b - a
                            ]
                            nc.tensor.matmul(
                                po,
                                yT_sb[:, i * 128 : (i + 1) * 128],
                                wo_sb[:, a:b],
                                start=True,
                                stop=True,
                            )
                            if i % 2 == 0:
                                nc.vector.tensor_copy(osb[:, s, a:b], po)
                            else:
                                nc.scalar.activation(
                                    out=osb[:, s, a:b],
                                    in_=po,
                                    func=mybir.ActivationFunctionType.Copy,
                                )
                    if p % 2 == 0:
                        nc.sync.dma_start(out=out_re[:, 2 * p : 2 * p + 2, :], in_=osb)
                    else:
                        nc.scalar.dma_start(
                            out=out_re[:, 2 * p : 2 * p + 2, :], in_=osb
                        )

    nc.finalize()
    return nc


def _host_consts():
    s = np.arange(128)[:, None]
    t = np.arange(128)[None, :]
    tri = (s <= t).astype(np.float32)
    htri = 0.5 * tri
    sel = np.zeros((8, L), dtype=np.float32)
    for i in range(8):
        sel[:i, i * 128 : (i + 1) * 128] = 0.5
    ones8 = np.zeros((128, 64), dtype=np.float32)
    for ch in range(8):
        ones8[:, ch * 8 + ch] = 1.0
    return tri, htri, sel, ones8


def kernel(hidden_states, Wq, Wk, Wv, Wo):
    global _compiled_nc, _last_in_maps
    hs = np.asarray(hidden_states, dtype=np.float32)[0]  # [L, D]
    Wq = np.asarray(Wq, dtype=np.float32)
    Wk = np.asarray(Wk, dtype=np.float32)
    Wv = np.asarray(Wv, dtype=np.float32)
    Wo = np.asarray(Wo, dtype=np.float32)

    if _compiled_nc is None:
        _compiled_nc = _build_nc()
    nc = _compiled_nc

    bf = ml_dtypes.bfloat16
    hsT = np.ascontiguousarray(hs.T).astype(bf)  # [D, L]
    tri, htri, sel, ones8 = _host_consts()

    in_maps = []
    for c in range(NCORE):
        heads = [2 * c, 2 * c + 1]
        wqk_c = np.zeros((D, 128), dtype=np.float32)
        wv_c = np.zeros((D, 130), dtype=np.float32)
        wo_c = np.zeros((128, D), dtype=np.float32)
        for hi, h in enumerate(heads):
            if h >= H:
                continue
            wqk_c[:, 32 * hi : 32 * hi + FD] = Wk[:, h * FD : (h + 1) * FD]
            wqk_c[:, 64 + 32 * hi : 64 + 32 * hi + FD] = Wq[:, h * FD : (h + 1) * FD]
            wv_c[:, 65 * hi : 65 * hi + HD] = Wv[:, h * HD : (h + 1) * HD]
            wo_c[64 * hi : 64 * hi + HD, :] = Wo[h * HD : (h + 1) * HD, :]
        win_c = np.zeros((128, NWIN), dtype=np.float32)
        # wqk: [768, 128] -> [6, 128p, 128c] -> win[p, kb*128+c]
        win_c[:, WQK0 : WQK0 + KB * 128] = (
            wqk_c.reshape(KB, 128, 128).transpose(1, 0, 2).reshape(128, KB * 128)
        )
        win_c[:, WV0 : WV0 + KB * 130] = (
            wv_c.reshape(KB, 128, 130).transpose(1, 0, 2).reshape(128, KB * 130)
        )
        win_c[:, WO0 : WO0 + D] = wo_c
        win_c[:, TRI2_0 : TRI2_0 + 128] = tri
        win_c[:, TRI2_0 + 128 : TRI2_0 + 256] = tri
        win_c[:, HTRI0 : HTRI0 + 128] = htri
        win_c[:, ONES8_0 : ONES8_0 + 64] = ones8
        in_maps.append(
            {
                "hsT": hsT,
                "win": win_c.astype(bf),
                "selw": sel.astype(bf),
            }
        )

    _last_in_maps = in_maps
    res = run_bass_kernel_spmd(nc, in_maps, list(range(NCORE)))
    acc = np.zeros((L, D), dtype=np.float32)
    for c in range(NCORE):
        acc += np.asarray(res.results[c]["outp"], dtype=np.float32)
    return acc.reshape(1, L, D)


# revision 4
# speedup vs baseline: 1.0535x; 1.0284x over previous
"""Based-style linear attention (Taylor feature map) on 8 Trainium2 cores.

Math: reference computes, per head h (FDIM=16, HEAD_DIM=64):
    q,k = HS@Wq, HS@Wk    (per-head 16 dims), v = HS@Wv (per-head 64 dims)
    phi(x) = [1, x/2, outer(x,x)/(sqrt(2)*4)]      (273 dims)
    y_t = sum_{s<=t} (phi(q_t).phi(k_s)) v_s / sum_{s<=t} phi(q_t).phi(k_s)
    out = concat_h(y) @ Wo

Key identity: phi(q).phi(k) = 1 + S/4 + S^2/32 where S = q.k (16-dim dot)
            = Square(S/sqrt(32) + 1/sqrt(2)) + 1/2.
So scores come from 16-dim dot products + one ACT Square pass; the 273-dim
feature map is never materialized.

Sharding: head-parallel, no collectives. 16 virtual heads (12 real + 4
zero dummies), 2 per core. Host sums the 8 partial outputs.

v3 structure (all per core, 2 heads):
 - inputs: one packed [128, NWIN] weights/consts tensor (2 DMAs) + hsT in
   12 half-L tiles (lo = t<512 first), issued in parallel from the two
   HWDGE queues (sync + scalar).
 - 12 dummy matmuls on zeros open the PE queue: they run during the input
   DMA wait and warm the PE HAM clock gate (cold 1.2GHz -> warm 2.4GHz).
 - q/k projections use ONE merged 128-col stationary (k_h0|k_h1|q_h0|q_h1
   16-col groups at 32-col offsets) so hs is streamed once, not twice.
 - attention runs in two passes over query columns (t<512, then t>=512)
   so pass A starts as soon as the lo half of hs has landed.
 - score matmuls have K=32: the two heads' stationaries sit at partition
   offsets 0/32 so they row-tile into independent 32x128 PE sub-arrays
   and run concurrently; both heads share one psum bank pair and one ACT
   Square call per (pass, j).
 - nums[h] [65, L] psum: v-cols 0-63 + den ride-along col 64 (ones), +1/2
   causal terms folded in via htri / colsum-sel constant matmuls.
 - divide, pipelined by L-half: reciprocal_approx_fast on nums, K=1
   matmul broadcasts the den-reciprocal row into a [128, L] psum, ACT
   copies -> rb, DVE muls produce yT [128, L] (h0 rows 0-63, h1 64-127).
 - o-proj: yT stacked layout makes it ONE K=128 matmul group per chunk
   (wo is [128, 768] with both heads' rows). psum->sbuf copies alternate
   ACT/DVE; output chunks pair into 4 DMAs on the idle sync queue.
"""

import math

import numpy as np
import ml_dtypes

import concourse.bass as bass
import concourse.mybir as mybir
import concourse.tile as tile
from concourse import bacc
from concourse.bass_utils import run_bass_kernel_spmd

L = 1024
D = 768
H = 12
FD = 16
HD = 64
NCORE = 8
NCH = 8  # L chunks of 128
KB = 6  # contraction blocks of 128 over D
F32 = mybir.dt.float32
BF16 = mybir.dt.bfloat16
DT = BF16

A_SCALE = 1.0 / math.sqrt(32.0)
A_BIAS = 1.0 / math.sqrt(2.0)

# win column map ([128, NWIN] bf16)
WQK0 = 0                 # 6 kb-blocks x 128 (merged qk stationary)
WV0 = WQK0 + KB * 128    # 6 kb-blocks x 130
WINA = WV0 + KB * 130    # end of win_a
WO0 = 0                  # win_b: wo [128, 768] (h0 rows 0-63, h1 64-127)
TRI2_0 = WO0 + D         # [tri | tri] 256
HTRI0 = TRI2_0 + 256     # htri 128
ONES8_0 = HTRI0 + 128    # ones8 64
NWINB = ONES8_0 + 64
NWIN = WINA + NWINB

_compiled_nc = None
_last_in_maps = None


def _build_nc():
    nc = bacc.Bacc("TRN2", target_bir_lowering=False, debug=False, num_devices=NCORE)

    hsT = nc.dram_tensor("hsT", [D, L], DT, kind="ExternalInput")
    win = nc.dram_tensor("win", [128, NWIN], DT, kind="ExternalInput")
    selw = nc.dram_tensor("selw", [8, L], DT, kind="ExternalInput")
    outp = nc.dram_tensor("outp", [L, D], DT, kind="ExternalOutput")

    with tile.TileContext(nc) as tc:
        with (
            tc.tile_pool(name="cst", bufs=1) as cst,
            tc.tile_pool(name="sqp", bufs=5) as sqp,
            tc.tile_pool(name="wrk", bufs=2) as wrk,
        ):
            # ---- PE warm-up: first in the PE queue, runs during DMA wait ----
            warm_sb = cst.tile([128, 512], DT, tag="warm")
            nc.vector.memset(warm_sb, 0.0)
            warm_out = cst.tile([128, 1], F32, tag="warmout")
            with tc.tile_pool(name="psw", bufs=1, space="PSUM") as psw:
                pw = psw.tile([128, 512], F32, tag="pw")
                for i in range(12):
                    nc.tensor.matmul(
                        pw, warm_sb[:, 0:128], warm_sb, start=(i == 0), stop=(i == 11)
                    )
                nc.vector.tensor_copy(warm_out, pw[:, 0:1])

            # ---- input DMAs: split across the two HWDGE queues ----
            hs_re = hsT.ap().rearrange("(po pi) f -> pi po f", pi=128)
            wina_sb = cst.tile([128, WINA], DT, tag="wina")
            winb_sb = cst.tile([128, NWINB], DT, tag="winb")
            sel_sb = cst.tile([8, L], DT, tag="sel")
            hs_lo = [
                cst.tile([128, 512], DT, tag=f"hslo{kb}", name=f"hslo{kb}")
                for kb in range(KB)
            ]
            hs_hi = [
                cst.tile([128, 512], DT, tag=f"hshi{kb}", name=f"hshi{kb}")
                for kb in range(KB)
            ]
            # sync queue
            nc.sync.dma_start(out=wina_sb, in_=win.ap()[:, 0:WINA])
            for kb in (0, 2, 4):
                nc.sync.dma_start(out=hs_lo[kb], in_=hs_re[:, kb, 0:512])
            for kb in (0, 2, 4):
                nc.sync.dma_start(out=hs_hi[kb], in_=hs_re[:, kb, 512:1024])
            # scalar queue
            for kb in (1, 3, 5):
                nc.scalar.dma_start(out=hs_lo[kb], in_=hs_re[:, kb, 0:512])
            nc.scalar.dma_start(out=winb_sb, in_=win.ap()[:, WINA:NWIN])
            for kb in (1, 3, 5):
                nc.scalar.dma_start(out=hs_hi[kb], in_=hs_re[:, kb, 512:1024])
            nc.scalar.dma_start(out=sel_sb, in_=selw.ap())

            def hs(kb, c0, c1):
                if c1 <= 512:
                    return hs_lo[kb][:, c0:c1]
                return hs_hi[kb][:, c0 - 512 : c1 - 512]

            def wqk(kb):
                return wina_sb[:, WQK0 + kb * 128 : WQK0 + (kb + 1) * 128]

            def wv(kb):
                return wina_sb[:, WV0 + kb * 130 : WV0 + (kb + 1) * 130]

            wo_sb = winb_sb[:, WO0 : WO0 + D]
            tri2_sb = winb_sb[:, TRI2_0 : TRI2_0 + 256]
            htri_sb = winb_sb[:, HTRI0 : HTRI0 + 128]
            ones8_sb = winb_sb[:, ONES8_0 : ONES8_0 + 64]

            bias_sb = cst.tile([128, 1], F32, tag="bias")
            nc.vector.memset(bias_sb, A_BIAS)
            # row of ones at partition 64, for the den-reciprocal broadcast
            ones64_sb = cst.tile([65, 64], F32, tag="ones64")
            nc.vector.memset(ones64_sb, 0.0)
            nc.vector.memset(ones64_sb[64:65, :], 1.0)

            kq_sb = cst.tile([64, 2048], DT, tag="kq")
            vx_sb = cst.tile([128, NCH, 130], DT, tag="vx")
            colsum_sb = cst.tile([8, 130], DT, tag="colsum")

            # ================= projections =================
            with tc.tile_pool(name="ps1", bufs=4, space="PSUM") as ps1:
                # q/k -> kq_sb [64, 2048]; partitions 0-15 head0, 32-47 head1
                # (rest zero); cols 0-1023 = k^T, 1024-2047 = q^T
                for half in range(2):
                    ph = ps1.tile([128, 512], F32, tag="pB", name=f"pqk{half}")
                    for kb in range(KB):
                        nc.tensor.matmul(
                            ph,
                            wqk(kb),
                            hs(kb, half * 512, (half + 1) * 512),
                            start=(kb == 0),
                            stop=(kb == KB - 1),
                        )
                    co = half * 512
                    nc.scalar.activation(
                        out=kq_sb[:, co : co + 512],
                        in_=ph[0:64, :],
                        func=mybir.ActivationFunctionType.Copy,
                    )
                    nc.vector.tensor_copy(
                        kq_sb[:, 1024 + co : 1024 + co + 512], ph[64:128, :]
                    )
                # v -> vx_sb [128, 8, 130]: cols 0-63 v_h0, 64 ones,
                # 65-128 v_h1, 129 ones
                for ch in range(NCH):
                    pv = ps1.tile([128, 130], F32, tag="pB", name=f"pv{ch}")
                    for kb in range(KB):
                        nc.tensor.matmul(
                            pv,
                            hs(kb, ch * 128, (ch + 1) * 128),
                            wv(kb),
                            start=(kb == 0),
                            stop=(kb == KB - 1),
                        )
                    nc.vector.tensor_copy(vx_sb[:, ch, :], pv)
                nc.vector.memset(vx_sb[:, :, 64], 1.0)
                nc.vector.memset(vx_sb[:, :, 129], 1.0)

                # per-chunk column sums of vx (inter-chunk +1/2 term)
                pcs = ps1.tile([8, 130], F32, tag="pB", name="pcs")
                for ch in range(NCH):
                    nc.tensor.matmul(
                        pcs,
                        ones8_sb[:, ch * 8 : (ch + 1) * 8],
                        vx_sb[:, ch, :],
                        start=(ch == 0),
                        stop=(ch == NCH - 1),
                    )
                nc.vector.tensor_copy(colsum_sb, pcs)

            # ================= attention =================
            yT_sb = cst.tile([128, L], DT, tag="yT")
            with tc.tile_pool(name="psnum", bufs=1, space="PSUM") as psnum:
                nums = [
                    psnum.tile([65, L], F32, tag=f"pN{h}", name=f"num{h}")
                    for h in range(2)
                ]
                sq_t = {}
                with tc.tile_pool(name="psa", bufs=2, space="PSUM") as psa:
                    # pass A: query cols t in [tlo, 512) for kv-chunks 0-3
                    for j in range(4):
                        tlo = j * 128
                        w = 512 - tlo
                        sq = sqp.tile([128, 2, L], DT, tag="sq", name=f"sq{j}")
                        sq_t[j] = sq
                        pa = psa.tile([128, 2, 512], F32, tag="pA", name=f"paA{j}")
                        for h in range(2):
                            nc.tensor.matmul(
                                pa[:, h, :w],
                                kq_sb[32 * h : 32 * h + 32, tlo : tlo + 128],
                                kq_sb[
                                    32 * h : 32 * h + 32, 1024 + tlo : 1024 + 512
                                ],
                                start=True,
                                stop=True,
                            )
                        nc.scalar.activation(
                            out=sq[:, :, :w],
                            in_=pa[:, :, :w],
                            func=mybir.ActivationFunctionType.Square,
                            scale=A_SCALE,
                            bias=bias_sb,
                        )
                        # mask the diagonal blocks (both heads, one op)
                        nc.vector.tensor_mul(
                            sq[:, :, 0:128], sq[:, :, 0:128], tri2_sb
                        )
                        for h in range(2):
                            nc.tensor.matmul(
                                nums[h][:, tlo:512],
                                vx_sb[:, j, 65 * h : 65 * h + 65],
                                sq[:, h, 0:w],
                                start=(j == 0),
                                stop=False,
                            )
                    # pass B: query cols t in [512, 1024) (j<4) or full (j>=4)
                    for j in range(NCH):
                        tlo = j * 128
                        if j < 4:
                            sq = sq_t[j]
                            c0, w = 512 - tlo, 512
                            qlo = 1024 + 512
                        else:
                            sq = sqp.tile([128, 2, L], DT, tag="sq", name=f"sq{j}")
                            c0, w = 0, L - tlo
                            qlo = 1024 + tlo
                        pa = psa.tile([128, 2, 512], F32, tag="pA", name=f"paB{j}")
                        for h in range(2):
                            nc.tensor.matmul(
                                pa[:, h, :w],
                                kq_sb[32 * h : 32 * h + 32, tlo : tlo + 128],
                                kq_sb[32 * h : 32 * h + 32, qlo : qlo + w],
                                start=True,
                                stop=True,
                            )
                        nc.scalar.activation(
                            out=sq[:, :, c0 : c0 + w],
                            in_=pa[:, :, :w],
                            func=mybir.ActivationFunctionType.Square,
                            scale=A_SCALE,
                            bias=bias_sb,
                        )
                        if j >= 4:
                            nc.vector.tensor_mul(
                                sq[:, :, 0:128], sq[:, :, 0:128], tri2_sb
                            )
                        for h in range(2):
                            nc.tensor.matmul(
                                nums[h][:, max(tlo, 512) : 1024],
                                vx_sb[:, j, 65 * h : 65 * h + 65],
                                sq[:, h, c0 : c0 + w],
                                start=(j == 0),
                                stop=False,
                            )
                    # +1/2 causal terms
                    for h in range(2):
                        for i in range(NCH):
                            nc.tensor.matmul(
                                nums[h][:, i * 128 : (i + 1) * 128],
                                vx_sb[:, i, 65 * h : 65 * h + 65],
                                htri_sb,
                                start=False,
                                stop=False,
                            )
                        for a, b in ((0, 512), (512, 1024)):
                            nc.tensor.matmul(
                                nums[h][:, a:b],
                                colsum_sb[:, 65 * h : 65 * h + 65],
                                sel_sb[:, a:b],
                                start=False,
                                stop=True,
                            )

                # y^T = num^T[0:64] / den  (den = row 64), pipelined by L-half
                with tc.tile_pool(name="ps2", bufs=1, space="PSUM") as ps2:
                    prb = ps2.tile([128, L], F32, tag="prb")
                    rb = wrk.tile([128, L], F32, tag="rb")
                    rcs = [
                        wrk.tile([65, L], F32, tag="rc", name=f"rc{h}")
                        for h in range(2)
                    ]
                    for half in range(2):
                        a, b = 512 * half, 512 * half + 512
                        for h in range(2):
                            # custom-DVE ops require base partition 0: run
                            # the approx reciprocal over all 65 rows (only
                            # den row 64 is used)
                            nc.vector.reciprocal_approx_fast(
                                out=rcs[h][:, a:b], in_=nums[h][:, a:b]
                            )
                            nc.tensor.matmul(
                                prb[64 * h : 64 * h + 64, a:b],
                                ones64_sb[64:65, :],
                                rcs[h][64:65, a:b],
                                start=True,
                                stop=True,
                            )
                        nc.scalar.activation(
                            out=rb[:, a:b],
                            in_=prb[:, a:b],
                            func=mybir.ActivationFunctionType.Copy,
                        )
                        for h in range(2):
                            nc.vector.tensor_mul(
                                yT_sb[64 * h : 64 * h + 64, a:b],
                                nums[h][0:64, a:b],
                                rb[64 * h : 64 * h + 64, a:b],
                            )

            # ================= output projection =================
            out_re = outp.ap().rearrange("(c p) d -> p c d", p=128)
            with tc.tile_pool(name="ps3", bufs=4, space="PSUM") as ps3:
                for p in range(NCH // 2):
                    osb = wrk.tile([128, 2, D], DT, tag="osb", name=f"osb{p}")
                    for s in range(2):
                        i = 2 * p + s
                        po = ps3.tile([128, D], F32, tag="po", name=f"po{i}")
                        for a, b in ((0, 512), (512, 768)):
                            nc.tensor.matmul(
                                po[:, a:b],
                                yT_sb[:, i * 128 : (i + 1) * 128],
                                wo_sb[:, a:b],
                                start=True,
                                stop=True,
                            )
                        if i % 2 == 0:
                            nc.vector.tensor_copy(osb[:, s, :], po)
                        else:
                            nc.scalar.activation(
                                out=osb[:, s, :],
                                in_=po,
                                func=mybir.ActivationFunctionType.Copy,
                            )
                    nc.sync.dma_start(out=out_re[:, 2 * p : 2 * p + 2, :], in_=osb)

    nc.finalize()
    return nc


def _host_consts():
    s = np.arange(128)[:, None]
    t = np.arange(128)[None, :]
    tri = (s <= t).astype(np.float32)
    htri = 0.5 * tri
    sel = np.zeros((8, L), dtype=np.float32)
    for i in range(8):
        sel[:i, i * 128 : (i + 1) * 128] = 0.5
    ones8 = np.zeros((128, 64), dtype=np.float32)
    for ch in range(8):
        ones8[:, ch * 8 + ch] = 1.0
    return tri, htri, sel, ones8


def kernel(hidden_states, Wq, Wk, Wv, Wo):
    global _compiled_nc, _last_in_maps
    hs = np.asarray(hidden_states, dtype=np.float32)[0]  # [L, D]
    Wq = np.asarray(Wq, dtype=np.float32)
    Wk = np.asarray(Wk, dtype=np.float32)
    Wv = np.asarray(Wv, dtype=np.float32)
    Wo = np.asarray(Wo, dtype=np.float32)

    if _compiled_nc is None:
        _compiled_nc = _build_nc()
    nc = _compiled_nc

    bf = ml_dtypes.bfloat16
    hsT = np.ascontiguousarray(hs.T).astype(bf)  # [D, L]
    tri, htri, sel, ones8 = _host_consts()

    in_maps = []
    for c in range(NCORE):
        heads = [2 * c, 2 * c + 1]
        wqk_c = np.zeros((D, 128), dtype=np.float32)
        wv_c = np.zeros((D, 130), dtype=np.float32)
        wo_c = np.zeros((128, D), dtype=np.float32)
        for hi, h in enumerate(heads):
            if h >= H:
                continue
            wqk_c[:, 32 * hi : 32 * hi + FD] = Wk[:, h * FD : (h + 1) * FD]
            wqk_c[:, 64 + 32 * hi : 64 + 32 * hi + FD] = Wq[:, h * FD : (h + 1) * FD]
            wv_c[:, 65 * hi : 65 * hi + HD] = Wv[:, h * HD : (h + 1) * HD]
            wo_c[64 * hi : 64 * hi + HD, :] = Wo[h * HD : (h + 1) * HD, :]
        win_c = np.zeros((128, NWIN), dtype=np.float32)
        # wqk: [768, 128] -> [6, 128p, 128c] -> win[p, kb*128+c]
        win_c[:, WQK0 : WQK0 + KB * 128] = (
            wqk_c.reshape(KB, 128, 128).transpose(1, 0, 2).reshape(128, KB * 128)
        )
        win_c[:, WV0 : WV0 + KB * 130] = (
            wv_c.reshape(KB, 128, 130).transpose(1, 0, 2).reshape(128, KB * 130)
        )
        wb = WINA
        win_c[:, wb + WO0 : wb + WO0 + D] = wo_c
        win_c[:, wb + TRI2_0 : wb + TRI2_0 + 128] = tri
        win_c[:, wb + TRI2_0 + 128 : wb + TRI2_0 + 256] = tri
        win_c[:, wb + HTRI0 : wb + HTRI0 + 128] = htri
        win_c[:, wb + ONES8_0 : wb + ONES8_0 + 64] = ones8
        in_maps.append(
            {
                "hsT": hsT,
                "win": win_c.astype(bf),
                "selw": sel.astype(bf),
            }
        )

    _last_in_maps = in_maps
    res = run_bass_kernel_spmd(nc, in_maps, list(range(NCORE)))
    acc = np.zeros((L, D), dtype=np.float32)
    for c in range(NCORE):
        acc += np.asarray(res.results[c]["outp"], dtype=np.float32)
    return acc.reshape(1, L, D)


# revision 10
# speedup vs baseline: 1.0599x; 1.0061x over previous
"""Based-style linear attention (Taylor feature map) on 8 Trainium2 cores.

Math: reference computes, per head h (FDIM=16, HEAD_DIM=64):
    q,k = HS@Wq, HS@Wk    (per-head 16 dims), v = HS@Wv (per-head 64 dims)
    phi(x) = [1, x/2, outer(x,x)/(sqrt(2)*4)]      (273 dims)
    y_t = sum_{s<=t} (phi(q_t).phi(k_s)) v_s / sum_{s<=t} phi(q_t).phi(k_s)
    out = concat_h(y) @ Wo

Key identity: phi(q).phi(k) = 1 + S/4 + S^2/32 where S = q.k (16-dim dot)
            = Square(S/sqrt(32) + 1/sqrt(2)) + 1/2.
So scores come from 16-dim dot products + one ACT Square pass; the 273-dim
feature map is never materialized.

Sharding: head-parallel, no collectives. 16 virtual heads (12 real + 4
zero dummies), 2 per core. Host sums the 8 partial outputs.

v3 structure (all per core, 2 heads):
 - inputs: one packed [128, NWIN] weights/consts tensor (2 DMAs) + hsT in
   12 half-L tiles (lo = t<512 first), issued in parallel from the two
   HWDGE queues (sync + scalar).
 - 12 dummy matmuls on zeros open the PE queue: they run during the input
   DMA wait and warm the PE HAM clock gate (cold 1.2GHz -> warm 2.4GHz).
 - q/k projections use ONE merged 128-col stationary (k_h0|k_h1|q_h0|q_h1
   16-col groups at 32-col offsets) so hs is streamed once, not twice.
 - attention runs in two passes over query columns (t<512, then t>=512)
   so pass A starts as soon as the lo half of hs has landed.
 - score matmuls have K=32: the two heads' stationaries sit at partition
   offsets 0/32 so they row-tile into independent 32x128 PE sub-arrays
   and run concurrently; both heads share one psum bank pair and one ACT
   Square call per (pass, j).
 - nums[h] [65, L] psum: v-cols 0-63 + den ride-along col 64 (ones), +1/2
   causal terms folded in via htri / colsum-sel constant matmuls.
 - divide, pipelined by L-half: reciprocal_approx_fast on nums, K=1
   matmul broadcasts the den-reciprocal row into a [128, L] psum, ACT
   copies -> rb, DVE muls produce yT [128, L] (h0 rows 0-63, h1 64-127).
 - o-proj: yT stacked layout makes it ONE K=128 matmul group per chunk
   (wo is [128, 768] with both heads' rows). psum->sbuf copies alternate
   ACT/DVE; output chunks pair into 4 DMAs on the idle sync queue.
"""

import math

import numpy as np
import ml_dtypes

import concourse.bass as bass
import concourse.mybir as mybir
import concourse.tile as tile
from concourse import bacc
from concourse.bass_utils import run_bass_kernel_spmd

L = 1024
D = 768
H = 12
FD = 16
HD = 64
NCORE = 8
NCH = 8  # L chunks of 128
KB = 6  # contraction blocks of 128 over D
F32 = mybir.dt.float32
BF16 = mybir.dt.bfloat16
DT = BF16

A_SCALE = 1.0 / math.sqrt(32.0)
A_BIAS = 1.0 / math.sqrt(2.0)

# win column map ([128, NWIN] bf16)
WQK0 = 0                 # 6 kb-blocks x 128 (merged qk stationary)
WV0 = WQK0 + KB * 128    # 6 kb-blocks x 130
WINA = WV0 + KB * 130    # end of win_a
WO0 = 0                  # win_b: wo [128, 768] (h0 rows 0-63, h1 64-127)
TRI2_0 = WO0 + D         # [tri | tri] 256
HTRI0 = TRI2_0 + 256     # htri 128
ONES8_0 = HTRI0 + 128    # ones8 64
NWINB = ONES8_0 + 64
NWIN = WINA + NWINB

_compiled_nc = None
_last_in_maps = None


def _build_nc():
    nc = bacc.Bacc("TRN2", target_bir_lowering=False, debug=False, num_devices=NCORE)

    hsT = nc.dram_tensor("hsT", [D, L], DT, kind="ExternalInput")
    win = nc.dram_tensor("win", [128, NWIN], DT, kind="ExternalInput")
    selw = nc.dram_tensor("selw", [8, L], DT, kind="ExternalInput")
    outp = nc.dram_tensor("outp", [L, D], DT, kind="ExternalOutput")

    with tile.TileContext(nc) as tc:
        with (
            tc.tile_pool(name="cst", bufs=1) as cst,
            tc.tile_pool(name="sqp", bufs=5) as sqp,
            tc.tile_pool(name="wrk", bufs=2) as wrk,
        ):
            # ---- PE warm-up: first in the PE queue, runs during DMA wait ----
            warm_sb = cst.tile([128, 512], DT, tag="warm")
            nc.vector.memset(warm_sb, 0.0)
            warm_out = cst.tile([128, 1], F32, tag="warmout")
            with tc.tile_pool(name="psw", bufs=1, space="PSUM") as psw:
                pw = psw.tile([128, 512], F32, tag="pw")
                for i in range(12):
                    nc.tensor.matmul(
                        pw, warm_sb[:, 0:128], warm_sb, start=(i == 0), stop=(i == 11)
                    )
                nc.vector.tensor_copy(warm_out, pw[:, 0:1])

            # ---- input DMAs: split across the two HWDGE queues ----
            hs_re = hsT.ap().rearrange("(po pi) f -> pi po f", pi=128)
            wina_sb = cst.tile([128, WINA], DT, tag="wina")
            winb_sb = cst.tile([128, NWINB], DT, tag="winb")
            sel_sb = cst.tile([8, L], DT, tag="sel")
            hs_lo = [
                cst.tile([128, 512], DT, tag=f"hslo{kb}", name=f"hslo{kb}")
                for kb in range(KB)
            ]
            hs_hi = [
                cst.tile([128, 512], DT, tag=f"hshi{kb}", name=f"hshi{kb}")
                for kb in range(KB)
            ]
            # sync queue
            nc.sync.dma_start(out=wina_sb, in_=win.ap()[:, 0:WINA])
            for kb in (0, 2, 4):
                nc.sync.dma_start(out=hs_lo[kb], in_=hs_re[:, kb, 0:512])
            for kb in (0, 2, 4):
                nc.sync.dma_start(out=hs_hi[kb], in_=hs_re[:, kb, 512:1024])
            # scalar queue
            for kb in (1, 3, 5):
                nc.scalar.dma_start(out=hs_lo[kb], in_=hs_re[:, kb, 0:512])
            nc.scalar.dma_start(out=winb_sb, in_=win.ap()[:, WINA:NWIN])
            for kb in (1, 3, 5):
                nc.scalar.dma_start(out=hs_hi[kb], in_=hs_re[:, kb, 512:1024])
            nc.scalar.dma_start(out=sel_sb, in_=selw.ap())

            def hs(kb, c0, c1):
                if c1 <= 512:
                    return hs_lo[kb][:, c0:c1]
                return hs_hi[kb][:, c0 - 512 : c1 - 512]

            def wqk(kb):
                return wina_sb[:, WQK0 + kb * 128 : WQK0 + (kb + 1) * 128]

            def wv(kb):
                return wina_sb[:, WV0 + kb * 130 : WV0 + (kb + 1) * 130]

            wo_sb = winb_sb[:, WO0 : WO0 + D]
            tri2_sb = winb_sb[:, TRI2_0 : TRI2_0 + 256]
            htri_sb = winb_sb[:, HTRI0 : HTRI0 + 128]
            ones8_sb = winb_sb[:, ONES8_0 : ONES8_0 + 64]

            bias_sb = cst.tile([128, 1], F32, tag="bias")
            nc.vector.memset(bias_sb, A_BIAS)
            # row of ones at partition 64, for the den-reciprocal broadcast
            ones64_sb = cst.tile([65, 64], F32, tag="ones64")
            nc.vector.memset(ones64_sb, 0.0)
            nc.vector.memset(ones64_sb[64:65, :], 1.0)

            kq_sb = cst.tile([64, 2048], DT, tag="kq")
            vx_sb = cst.tile([128, NCH, 130], DT, tag="vx")
            colsum_sb = cst.tile([8, 130], DT, tag="colsum")

            # ================= projections =================
            with tc.tile_pool(name="ps1", bufs=4, space="PSUM") as ps1:
                # q/k -> kq_sb [64, 2048]; partitions 0-15 head0, 32-47 head1
                # (rest zero); cols 0-1023 = k^T, 1024-2047 = q^T
                for half in range(2):
                    ph = ps1.tile([128, 512], F32, tag="pB", name=f"pqk{half}")
                    for kb in range(KB):
                        nc.tensor.matmul(
                            ph,
                            wqk(kb),
                            hs(kb, half * 512, (half + 1) * 512),
                            start=(kb == 0),
                            stop=(kb == KB - 1),
                        )
                    co = half * 512
                    nc.scalar.activation(
                        out=kq_sb[:, co : co + 512],
                        in_=ph[0:64, :],
                        func=mybir.ActivationFunctionType.Copy,
                    )
                    nc.vector.tensor_copy(
                        kq_sb[:, 1024 + co : 1024 + co + 512], ph[64:128, :]
                    )
                # v -> vx_sb [128, 8, 130]: cols 0-63 v_h0, 64 ones,
                # 65-128 v_h1, 129 ones
                for ch in range(NCH):
                    pv = ps1.tile([128, 130], F32, tag="pB", name=f"pv{ch}")
                    for kb in range(KB):
                        nc.tensor.matmul(
                            pv,
                            hs(kb, ch * 128, (ch + 1) * 128),
                            wv(kb),
                            start=(kb == 0),
                            stop=(kb == KB - 1),
                        )
                    nc.vector.tensor_copy(vx_sb[:, ch, :], pv)
                nc.vector.memset(vx_sb[:, :, 64], 1.0)
                nc.vector.memset(vx_sb[:, :, 129], 1.0)

                # per-chunk column sums of vx (inter-chunk +1/2 term)
                pcs = ps1.tile([8, 130], F32, tag="pB", name="pcs")
                for ch in range(NCH):
                    nc.tensor.matmul(
                        pcs,
                        ones8_sb[:, ch * 8 : (ch + 1) * 8],
                        vx_sb[:, ch, :],
                        start=(ch == 0),
                        stop=(ch == NCH - 1),
                    )
                nc.vector.tensor_copy(colsum_sb, pcs)

            # ================= attention =================
            yT_sb = cst.tile([128, L], DT, tag="yT")
            with tc.tile_pool(name="psnum", bufs=1, space="PSUM") as psnum:
                nums = [
                    psnum.tile([65, L], F32, tag=f"pN{h}", name=f"num{h}")
                    for h in range(2)
                ]
                sq_t = {}
                with tc.tile_pool(name="psa", bufs=2, space="PSUM") as psa:
                    # pass A: query cols t in [tlo, 512) for kv-chunks 0-3
                    for j in range(4):
                        tlo = j * 128
                        w = 512 - tlo
                        sq = sqp.tile([128, 2, L], DT, tag="sq", name=f"sq{j}")
                        sq_t[j] = sq
                        pa = psa.tile([128, 2, 512], F32, tag="pA", name=f"paA{j}")
                        for h in range(2):
                            nc.tensor.matmul(
                                pa[:, h, :w],
                                kq_sb[32 * h : 32 * h + 32, tlo : tlo + 128],
                                kq_sb[
                                    32 * h : 32 * h + 32, 1024 + tlo : 1024 + 512
                                ],
                                start=True,
                                stop=True,
                            )
                        nc.scalar.activation(
                            out=sq[:, :, :w],
                            in_=pa[:, :, :w],
                            func=mybir.ActivationFunctionType.Square,
                            scale=A_SCALE,
                            bias=bias_sb,
                        )
                        # diagonal blocks: fold the +1/2 term and the causal
                        # mask into one op: sq' = (sq + 0.5) * tri
                        nc.vector.scalar_tensor_tensor(
                            out=sq[:, :, 0:128],
                            in0=sq[:, :, 0:128],
                            scalar=0.5,
                            in1=tri2_sb,
                            op0=mybir.AluOpType.add,
                            op1=mybir.AluOpType.mult,
                        )
                        for h in range(2):
                            nc.tensor.matmul(
                                nums[h][:, tlo:512],
                                vx_sb[:, j, 65 * h : 65 * h + 65],
                                sq[:, h, 0:w],
                                start=(j == 0),
                                stop=False,
                            )
                    # pass B: query cols t in [512, 1024) (j<4) or full (j>=4)
                    for j in range(NCH):
                        tlo = j * 128
                        if j < 4:
                            sq = sq_t[j]
                            c0, w = 512 - tlo, 512
                            qlo = 1024 + 512
                        else:
                            sq = sqp.tile([128, 2, L], DT, tag="sq", name=f"sq{j}")
                            c0, w = 0, L - tlo
                            qlo = 1024 + tlo
                        pa = psa.tile([128, 2, 512], F32, tag="pA", name=f"paB{j}")
                        for h in range(2):
                            nc.tensor.matmul(
                                pa[:, h, :w],
                                kq_sb[32 * h : 32 * h + 32, tlo : tlo + 128],
                                kq_sb[32 * h : 32 * h + 32, qlo : qlo + w],
                                start=True,
                                stop=True,
                            )
                        nc.scalar.activation(
                            out=sq[:, :, c0 : c0 + w],
                            in_=pa[:, :, :w],
                            func=mybir.ActivationFunctionType.Square,
                            scale=A_SCALE,
                            bias=bias_sb,
                        )
                        if j >= 4:
                            nc.vector.scalar_tensor_tensor(
                                out=sq[:, :, 0:128],
                                in0=sq[:, :, 0:128],
                                scalar=0.5,
                                in1=tri2_sb,
                                op0=mybir.AluOpType.add,
                                op1=mybir.AluOpType.mult,
                            )
                        for h in range(2):
                            nc.tensor.matmul(
                                nums[h][:, max(tlo, 512) : 1024],
                                vx_sb[:, j, 65 * h : 65 * h + 65],
                                sq[:, h, c0 : c0 + w],
                                start=(j == 0),
                                stop=False,
                            )
                    # inter-chunk +1/2 term: 0.5 * sum of prior chunk colsums
                    # (the intra-chunk +1/2 is folded into the diag mask op)
                    for h in range(2):
                        for a, b in ((0, 512), (512, 1024)):
                            nc.tensor.matmul(
                                nums[h][:, a:b],
                                colsum_sb[:, 65 * h : 65 * h + 65],
                                sel_sb[:, a:b],
                                start=False,
                                stop=True,
                            )

                # y^T = num^T[0:64] / den  (den = row 64), pipelined by L-half
                with tc.tile_pool(name="ps2", bufs=1, space="PSUM") as ps2:
                    # dummy matmuls keep the PE HAM clock gate warm while
                    # the DVE reciprocal runs
                    pwd = ps2.tile([128, 512], F32, tag="pwd")
                    for i in range(8):
                        nc.tensor.matmul(
                            pwd,
                            warm_sb[:, 0:128],
                            warm_sb,
                            start=(i == 0),
                            stop=(i == 7),
                        )
                    nc.vector.tensor_copy(warm_out, pwd[:, 0:1])
                    prb = ps2.tile([128, L], F32, tag="prb")
                    rb = wrk.tile([128, L], F32, tag="rb")
                    rcs = [
                        wrk.tile([65, L], F32, tag="rc", name=f"rc{h}")
                        for h in range(2)
                    ]
                    for half in range(2):
                        a, b = 512 * half, 512 * half + 512
                        for h in range(2):
                            # custom-DVE ops require base partition 0: run
                            # the approx reciprocal over all 65 rows (only
                            # den row 64 is used)
                            nc.vector.reciprocal_approx_fast(
                                out=rcs[h][:, a:b], in_=nums[h][:, a:b]
                            )
                            nc.tensor.matmul(
                                prb[64 * h : 64 * h + 64, a:b],
                                ones64_sb[64:65, :],
                                rcs[h][64:65, a:b],
                                start=True,
                                stop=True,
                            )
                        nc.scalar.activation(
                            out=rb[:, a:b],
                            in_=prb[:, a:b],
                            func=mybir.ActivationFunctionType.Copy,
                        )
                        for h in range(2):
                            nc.vector.tensor_mul(
                                yT_sb[64 * h : 64 * h + 64, a:b],
                                nums[h][0:64, a:b],
                                rb[64 * h : 64 * h + 64, a:b],
                            )

            # ================= output projection =================
            out_re = outp.ap().rearrange("(c p) d -> p c d", p=128)
            with tc.tile_pool(name="ps3", bufs=4, space="PSUM") as ps3:
                for p in range(NCH // 2):
                    osb = wrk.tile([128, 2, D], DT, tag="osb", name=f"osb{p}")
                    for s in range(2):
                        i = 2 * p + s
                        po = ps3.tile([128, D], F32, tag="po", name=f"po{i}")
                        for a, b in ((0, 512), (512, 768)):
                            nc.tensor.matmul(
                                po[:, a:b],
                                yT_sb[:, i * 128 : (i + 1) * 128],
                                wo_sb[:, a:b],
                                start=True,
                                stop=True,
                            )
                        if i in (1, 5):
                            nc.scalar.activation(
                                out=osb[:, s, :],
                                in_=po,
                                func=mybir.ActivationFunctionType.Copy,
                            )
                        else:
                            nc.vector.tensor_copy(osb[:, s, :], po)
                    nc.sync.dma_start(out=out_re[:, 2 * p : 2 * p + 2, :], in_=osb)

    nc.finalize()
    return nc


def _host_consts():
    s = np.arange(128)[:, None]
    t = np.arange(128)[None, :]
    tri = (s <= t).astype(np.float32)
    htri = 0.5 * tri
    sel = np.zeros((8, L), dtype=np.float32)
    for i in range(8):
        sel[:i, i * 128 : (i + 1) * 128] = 0.5
    ones8 = np.zeros((128, 64), dtype=np.float32)
    for ch in range(8):
        ones8[:, ch * 8 + ch] = 1.0
    return tri, htri, sel, ones8


def kernel(hidden_states, Wq, Wk, Wv, Wo):
    global _compiled_nc, _last_in_maps
    hs = np.asarray(hidden_states, dtype=np.float32)[0]  # [L, D]
    Wq = np.asarray(Wq, dtype=np.float32)
    Wk = np.asarray(Wk, dtype=np.float32)
    Wv = np.asarray(Wv, dtype=np.float32)
    Wo = np.asarray(Wo, dtype=np.float32)

    if _compiled_nc is None:
        _compiled_nc = _build_nc()
    nc = _compiled_nc

    bf = ml_dtypes.bfloat16
    hsT = np.ascontiguousarray(hs.T).astype(bf)  # [D, L]
    tri, htri, sel, ones8 = _host_consts()

    in_maps = []
    for c in range(NCORE):
        heads = [2 * c, 2 * c + 1]
        wqk_c = np.zeros((D, 128), dtype=np.float32)
        wv_c = np.zeros((D, 130), dtype=np.float32)
        wo_c = np.zeros((128, D), dtype=np.float32)
        for hi, h in enumerate(heads):
            if h >= H:
                continue
            wqk_c[:, 32 * hi : 32 * hi + FD] = Wk[:, h * FD : (h + 1) * FD]
            wqk_c[:, 64 + 32 * hi : 64 + 32 * hi + FD] = Wq[:, h * FD : (h + 1) * FD]
            wv_c[:, 65 * hi : 65 * hi + HD] = Wv[:, h * HD : (h + 1) * HD]
            wo_c[64 * hi : 64 * hi + HD, :] = Wo[h * HD : (h + 1) * HD, :]
        win_c = np.zeros((128, NWIN), dtype=np.float32)
        # wqk: [768, 128] -> [6, 128p, 128c] -> win[p, kb*128+c]
        win_c[:, WQK0 : WQK0 + KB * 128] = (
            wqk_c.reshape(KB, 128, 128).transpose(1, 0, 2).reshape(128, KB * 128)
        )
        win_c[:, WV0 : WV0 + KB * 130] = (
            wv_c.reshape(KB, 128, 130).transpose(1, 0, 2).reshape(128, KB * 130)
        )
        wb = WINA
        win_c[:, wb + WO0 : wb + WO0 + D] = wo_c
        win_c[:, wb + TRI2_0 : wb + TRI2_0 + 128] = tri
        win_c[:, wb + TRI2_0 + 128 : wb + TRI2_0 + 256] = tri
        win_c[:, wb + HTRI0 : wb + HTRI0 + 128] = htri
        win_c[:, wb + ONES8_0 : wb + ONES8_0 + 64] = ones8
        in_maps.append(
            {
                "hsT": hsT,
                "win": win_c.astype(bf),
                "selw": sel.astype(bf),
            }
        )

    _last_in_maps = in_maps
    res = run_bass_kernel_spmd(nc, in_maps, list(range(NCORE)))
    acc = np.zeros((L, D), dtype=np.float32)
    for c in range(NCORE):
        acc += np.asarray(res.results[c]["outp"], dtype=np.float32)
    return acc.reshape(1, L, D)
